# revision 1
# baseline (speedup 1.0000x reference)
"""Trainium2 Bass kernel for a DeformableTransformerDecoderLayer.

Sharding: 8 cores = (batch b in 0..3) x (query-half in 0..1). Each core
processes 450 queries of one batch end-to-end (self-attn + MSDeformAttn +
FFN) with no collectives; the deformable value projection is computed per
batch on both cores of the pair (duplicated, cheaper than a collective).

Per-core pipeline (layouts chosen so no big on-device transposes are
needed; the host ships pre-transposed weights/activations):
  1. value = memory[b] @ vproj.T + b  -> DRAM [S, 256]  (PE, streamed)
  2. self-attn, transposed-score formulation: kT/qT [d, seq] tiles;
     scores^T [kj, qi] per head; exp without max-subtraction (logits are
     tiny for this module); softmax denominator via an appended ones
     column in the AV matmul; divide by a PE-broadcast reciprocal row.
  3. residual + LN2 (natural [q, d] rows, bn_stats/bn_aggr)
  4. sampling offsets / attention weights; tap index+weight pipeline (DVE)
  5. gather: one indirect row-gather DMA (32 floats per tap) per (head,
     query-tile); 64 taps per (q, h)
  6. weighted tap reduction on DVE; oproj via per-head K=32 matmuls
  7. residual + LN1, FFN (ff1 computed transposed so ff2 needs no
     transpose), residual + LN3, DMA out.
"""

import math
import ml_dtypes
import numpy as np

import concourse.bass as bass
import concourse.bacc as bacc
import concourse.tile as tile
from concourse import mybir
from concourse.bass_utils import run_bass_kernel_spmd
from concourse.masks import make_identity

D = 256; NH = 8; NL = 4; NPT = 4; DH = 32; DFFN = 1024; NQ = 900; BS = 4
SPATIAL = ((92, 92), (46, 46), (23, 23), (12, 12))
LEVEL_START = (0, 8464, 10580, 11109)
S = 11253
SPAD = 11264          # padded S (multiple of 128)
QH = 450              # queries per core
QPAD = 512            # padded queries per core
NKPAD = 1024          # padded key count (self-attn)
NKT = NKPAD // 128    # key tiles
NQT = QPAD // 128     # query tiles
NTAP = 64             # taps per (q, h): 4 levels * 4 points * 2 dy * 2 dx
TAPW = NH * NTAP      # 512
F32 = mybir.dt.float32
I32 = mybir.dt.int32
AO = mybir.AluOpType
AF = mybir.ActivationFunctionType

BF16 = mybir.dt.bfloat16
MM_DT = BF16          # matmul operand dtype (fp32 PSUM accumulation)


def _r(ap):
    return ap


def _v(a, ap_list, extra_offset=0):
    """Custom AP over the same tensor as AP `a`."""
    return bass.AP(tensor=a.tensor, offset=a.offset + extra_offset, ap=ap_list)


def _bc(a, n):
    """Append a broadcast (step-0) innermost dim of size n to AP `a`."""
    return bass.AP(tensor=a.tensor, offset=a.offset, ap=list(a.ap) + [[0, n]])


def _layernorm(nc, pool, x, out_ap, g_s, b_s, eps_s):
    """out = (x - mean)/sqrt(var+eps) * g + b over the free dim (256)."""
    st = pool.tile([128, 6], F32, tag="ln_st")
    nc.vector.bn_stats(out=st[:], in_=x)
    mv = pool.tile([128, 2], F32, tag="ln_mv")
    nc.vector.bn_aggr(out=mv[:], in_=st[:])
    rstd = pool.tile([128, 1], F32, tag="ln_rstd")
    nc.scalar.activation(out=rstd[:], in_=mv[:, 1:2], func=AF.Sqrt,
                         bias=eps_s[:], scale=1.0)
    nc.vector.reciprocal(out=rstd[:], in_=rstd[:])
    nc.vector.tensor_scalar(out=out_ap, in0=x, scalar1=mv[:, 0:1],
                            scalar2=rstd[:], op0=AO.subtract, op1=AO.mult)
    nc.vector.tensor_tensor(out=out_ap, in0=out_ap, in1=g_s[:], op=AO.mult)
    nc.vector.tensor_tensor(out=out_ap, in0=out_ap, in1=b_s[:], op=AO.add)


def build_program():
    nc = bacc.Bacc("TRN2", target_bir_lowering=False, debug=False)

    def inp(name, shape, dt=F32):
        return nc.declare_dram_parameter(name, list(shape), dt, isOutput=False)

    # activations (per-core shards; [128, kt, X] = K-tiled transposed layouts)
    tgtbT = inp("tgtbT", (128, 2, NKPAD), BF16)   # tgt[:,b,:].T, zero-padded
    posbT = inp("posbT", (128, 2, NKPAD), BF16)
    tgtb_ownT = inp("tgtb_ownT", (128, 2, QPAD), BF16)
    posb_ownT = inp("posb_ownT", (128, 2, QPAD), BF16)
    tgtb_own = inp("tgtb_own", (NQT, 128, D))  # own rows, natural
    pos_own = inp("pos_own", (NQT, 128, D))
    ref_own = inp("ref_own", (NQT, 128, NL * 2))
    memT = inp("memT", (128, 2, SPAD), BF16)         # memory[:,b,:].T

    # weights (pre-transposed / tiled on host)
    wqT = inp("wqT", (128, 2, D), BF16); wkT = inp("wkT", (128, 2, D), BF16); wvT = inp("wvT", (128, 2, D), BF16)
    bqp = inp("bqp", (128, 2)); bkp = inp("bkp", (128, 2))
    bvc = inp("bvc", (1, D))
    outwT8 = inp("outwT8", (32, NH * D), BF16); boutc = inp("boutc", (1, D))
    vprojwT = inp("vprojwT", (128, 2, D), BF16); bvpc = inp("bvpc", (1, D))
    offwT = inp("offwT", (128, 2, D), BF16); boffc = inp("boffc", (1, D))
    awwT = inp("awwT", (128, 2, NH * 16), BF16); bawc = inp("bawc", (1, NH * 16))
    oprojwT8 = inp("oprojwT8", (32, NH * D), BF16); bopc = inp("bopc", (1, D))
    lin1wT = inp("lin1wT", (128, 2, DFFN), BF16); b1col = inp("b1col", (128, DFFN // 128))
    lin2wT = inp("lin2wT", (128, 8, D), BF16); b2c = inp("b2c", (1, D))
    ln2g = inp("ln2g", (1, D)); ln2b = inp("ln2b", (1, D))
    ln1g = inp("ln1g", (1, D)); ln1b = inp("ln1b", (1, D))
    ln3g = inp("ln3g", (1, D)); ln3b = inp("ln3b", (1, D))

    # tap-grid constants (slot = h*64 + l*16 + p*4 + dy*2 + dx)
    cwh = inp("cwh", (1, NL * 2))
    cDx = inp("cDx", (1, TAPW)); cDy = inp("cDy", (1, TAPW))
    cDxu = inp("cDxu", (1, TAPW), mybir.dt.uint8)
    cDyu = inp("cDyu", (1, TAPW), mybir.dt.uint8)
    cW = inp("cW", (1, TAPW)); cH = inp("cH", (1, TAPW))
    cWm1 = inp("cWm1", (1, TAPW)); cHm1 = inp("cHm1", (1, TAPW))
    cW8 = inp("cW8", (1, TAPW)); cB8h = inp("cB8h", (1, TAPW))

    out = nc.declare_dram_parameter("out", [NQT, 128, D], F32, isOutput=True)
    import os as _os
    DBG = _os.environ.get("KDBG", "0") == "1"
    if DBG:
        dbg = nc.declare_dram_parameter("dbg", [NQT, 128, D], F32, isOutput=True)
        dbg2 = nc.declare_dram_parameter("dbg2", [NQT, 128, D], F32, isOutput=True)
        dbg3 = nc.declare_dram_parameter("dbg3", [NQT, 128, D], F32, isOutput=True)
        dbg4 = nc.declare_dram_parameter("dbg4", [NQT, 128, NH * 16], F32, isOutput=True)
        dbg5 = nc.declare_dram_parameter("dbg5", [NQT, 128, NH * DH], F32, isOutput=True)

    with tile.TileContext(nc) as tc:
        with (
            tc.tile_pool(name="sing", bufs=1) as sing,
            tc.tile_pool(name="stream", bufs=1) as stream,
            tc.tile_pool(name="dram", bufs=1, space="DRAM") as dpool,
            tc.tile_pool(name="work", bufs=2) as work,
            tc.tile_pool(name="mstream", bufs=3) as mstream,
            tc.tile_pool(name="vout", bufs=3) as vout,
        ):
            # ---------------- weights / constants into SBUF ----------------
            def load(t, shape, dt=None):
                s = sing.tile(list(shape), dt or t[:].dtype, tag="ld_" + t.name)
                nc.sync.dma_start(out=s[:], in_=t[:])
                return s

            def load_bcast(t, width):
                s = sing.tile([128, width], F32, tag="bc_" + t.name)
                nc.sync.dma_start(out=s[:], in_=_v(t[:], [[0, 128], [1, width]]))
                return s

            wq_s = load(wqT, (128, 2, D)); wk_s = load(wkT, (128, 2, D))
            wv_s = load(wvT, (128, 2, D))
            bq_s = load(bqp, (128, 2)); bk_s = load(bkp, (128, 2))
            bvc_s = load_bcast(bvc, D)
            outw_s = load(outwT8, (32, NH, D)); boutc_s = load_bcast(boutc, D)
            vpw_s = load(vprojwT, (128, 2, D)); bvpc_s = load_bcast(bvpc, D)
            offw_s = load(offwT, (128, 2, D)); boffc_s = load_bcast(boffc, D)
            aww_s = load(awwT, (128, 2, NH * 16)); bawc_s = load_bcast(bawc, NH * 16)
            opw_s = load(oprojwT8, (32, NH, D)); bopc_s = load_bcast(bopc, D)
            l1w_s = load(lin1wT, (128, 2, DFFN)); b1col_s = load(b1col, (128, DFFN // 128))
            l2w_s = load(lin2wT, (128, 8, D)); b2c_s = load_bcast(b2c, D)
            ln2g_s = load_bcast(ln2g, D); ln2b_s = load_bcast(ln2b, D)
            ln1g_s = load_bcast(ln1g, D); ln1b_s = load_bcast(ln1b, D)
            ln3g_s = load_bcast(ln3g, D); ln3b_s = load_bcast(ln3b, D)
            cwh_s = load_bcast(cwh, NL * 2)
            cDx_s = load_bcast(cDx, TAPW); cDy_s = load_bcast(cDy, TAPW)
            cDxu_s = sing.tile([128, TAPW], mybir.dt.uint8, tag="bc_cDxu")
            nc.sync.dma_start(out=cDxu_s[:], in_=_v(cDxu[:], [[0, 128], [1, TAPW]]))
            cDyu_s = sing.tile([128, TAPW], mybir.dt.uint8, tag="bc_cDyu")
            nc.sync.dma_start(out=cDyu_s[:], in_=_v(cDyu[:], [[0, 128], [1, TAPW]]))
            cW_s = load_bcast(cW, TAPW); cH_s = load_bcast(cH, TAPW)
            cWm1_s = load_bcast(cWm1, TAPW); cHm1_s = load_bcast(cHm1, TAPW)
            cW8_s = load_bcast(cW8, TAPW); cB8h_s = load_bcast(cB8h, TAPW)

            ident = sing.tile([128, 128], F32)
            make_identity(nc, ident[:])
            eps_s = sing.tile([128, 1], F32)
            nc.vector.memset(eps_s[:], 1e-5)
            ones32 = sing.tile([64, 32], F32)
            nc.vector.memset(ones32[:], 1.0)

            value = dpool.tile([SPAD, D], BF16)   # projected value (DRAM)

            # long-lived activation streams
            tgt2 = stream.tile([128, NQT, D], F32)   # post-LN2 (natural)
            x2T = stream.tile([128, 2, QPAD], BF16)   # (tgt2 + pos).T
            oD = stream.tile([128, NQT, NH, DH], F32)  # deform samples [q,h,d]

            # ---------------- value projection ----------------
            with tc.tile_pool(name="psVP", bufs=2, space="PSUM") as psVP:
                for st_ in range(SPAD // 128):
                    mem_sb = mstream.tile([128, 2, 128], BF16, tag="mem")
                    nc.sync.dma_start(out=mem_sb[:],
                                      in_=memT[:, :, st_ * 128:(st_ + 1) * 128])
                    vp = psVP.tile([128, D], F32, tag="vp")
                    for ki in range(2):
                        nc.tensor.matmul(out=vp[:], lhsT=_r(mem_sb[:, ki, :]),
                                         rhs=_r(vpw_s[:, ki, :]),
                                         start=(ki == 0), stop=(ki == 1))
                    v_sb = vout.tile([128, D], BF16, tag="v_sb")
                    nc.vector.tensor_tensor(out=v_sb[:], in0=vp[:], in1=bvpc_s[:],
                                            op=AO.add)
                    nc.sync.dma_start(out=value[st_ * 128:(st_ + 1) * 128, :],
                                      in_=v_sb[:])

                # ---------------- self-attention ----------------
                with (
                    tc.tile_pool(name="sa", bufs=1) as sa,
                    tc.tile_pool(name="epool", bufs=3) as epool,
                    tc.tile_pool(name="psAT", bufs=2, space="PSUM") as psAT,
                    tc.tile_pool(name="psAV", bufs=1, space="PSUM") as psAV,
                ):
                    tg_sb = sa.tile([128, 2, NKPAD], BF16)
                    po_sb = sa.tile([128, 2, NKPAD], BF16)
                    tgq_sb = sa.tile([128, 2, QPAD], BF16)
                    poq_sb = stream.tile([128, 2, QPAD], BF16)
                    nc.sync.dma_start(out=tg_sb[:], in_=tgtbT[:])
                    nc.sync.dma_start(out=po_sb[:], in_=posbT[:])
                    nc.sync.dma_start(out=tgq_sb[:], in_=tgtb_ownT[:])
                    nc.sync.dma_start(out=poq_sb[:], in_=posb_ownT[:])

                    kT = sa.tile([128, 2, NKPAD], BF16)
                    qT = sa.tile([128, 2, QPAD], BF16)
                    v_aug = sa.tile([128, NKT, NH, DH + 1], BF16)
                    oT = sa.tile([32, NH, QPAD], BF16)

                    for mi in range(2):
                        for nj in range(2):
                            ps = psAT.tile([128, 512], F32, tag="proj")
                            for si, srcb in enumerate((tg_sb, po_sb)):
                                for ki in range(2):
                                    nc.tensor.matmul(
                                        out=ps[:],
                                        lhsT=wk_s[:, ki, mi * 128:(mi + 1) * 128],
                                        rhs=srcb[:, ki, nj * 512:(nj + 1) * 512],
                                        start=(si == 0 and ki == 0),
                                        stop=(si == 1 and ki == 1))
                            nc.vector.tensor_scalar(
                                out=kT[:, mi, nj * 512:(nj + 1) * 512], in0=ps[:],
                                scalar1=bk_s[:, mi:mi + 1], scalar2=None, op0=AO.add)
                        psq = psAT.tile([128, 512], F32, tag="proj")
                        for si, srcb in enumerate((tgq_sb, poq_sb)):
                            for ki in range(2):
                                nc.tensor.matmul(
                                    out=psq[:],
                                    lhsT=wq_s[:, ki, mi * 128:(mi + 1) * 128],
                                    rhs=srcb[:, ki, :],
                                    start=(si == 0 and ki == 0),
                                    stop=(si == 1 and ki == 1))
                        nc.vector.tensor_scalar(
                            out=qT[:, mi, :], in0=psq[:],
                            scalar1=bq_s[:, mi:mi + 1], scalar2=None, op0=AO.add)

                    # v natural [kj, d] -> v_aug[.., h, 0:32]; ones col
                    for kjt in range(NKT):
                        psv = psAT.tile([128, D], F32, tag="proj")
                        for ki in range(2):
                            nc.tensor.matmul(
                                out=psv[:],
                                lhsT=tg_sb[:, ki, kjt * 128:(kjt + 1) * 128],
                                rhs=wv_s[:, ki, :],
                                start=(ki == 0), stop=(ki == 1))
                        nc.vector.tensor_tensor(
                            out=v_aug[:, kjt, :, 0:DH],
                            in0=psv[:].rearrange("p (h d) -> p h d", h=NH),
                            in1=bvc_s[:].rearrange("p (h d) -> p h d", h=NH),
                            op=AO.add)
                    nc.vector.memset(v_aug[:, :, :, DH:DH + 1], 1.0)

                    for h in range(NH):
                        mt, pt = h // 4, (h % 4) * 32
                        av = psAV.tile([DH + 1, QPAD], F32, tag="av")
                        for kjt in range(NKT):
                            sc = psAT.tile([128, QPAD], F32, tag="sc")
                            nc.tensor.matmul(
                                out=sc[:],
                                lhsT=_r(kT[pt:pt + 32, mt, kjt * 128:(kjt + 1) * 128]),
                                rhs=_r(qT[pt:pt + 32, mt, :]),
                                start=True, stop=True, tile_position=(pt, 0))
                            e_t = epool.tile([128, QPAD], BF16, tag="e")
                            if kjt == NKT - 1:  # keys 900.. are padding: e = 0
                                nc.vector.memset(e_t[:], 0.0)
                                nc.scalar.activation(out=e_t[0:NQ - 7 * 128, :],
                                                     in_=sc[0:NQ - 7 * 128, :],
                                                     func=AF.Exp)
                            else:
                                nc.scalar.activation(out=e_t[:], in_=sc[:],
                                                     func=AF.Exp)
                            nc.tensor.matmul(
                                out=av[:], lhsT=_r(v_aug[:, kjt, h, :]), rhs=_r(e_t[:]),
                                start=(kjt == 0), stop=(kjt == NKT - 1))
                        rd = work.tile([33, QPAD], F32, tag="rd")
                        nc.vector.reciprocal(out=rd[32:33, :], in_=av[32:33, :])
                        rbc = psAV.tile([32, QPAD], F32, tag="rbc")
                        nc.tensor.matmul(out=rbc[:], lhsT=ones32[32:33, :],
                                         rhs=rd[32:33, :], start=True, stop=True,
                                         tile_position=(32, 0))
                        rb_sb = work.tile([32, QPAD], F32, tag="rb_sb")
                        nc.vector.tensor_copy(out=rb_sb[:], in_=rbc[:])
                        nc.vector.tensor_tensor(out=oT[:, h, :], in0=av[0:DH, :],
                                                in1=rb_sb[:], op=AO.mult)

                    # out-proj + residual + LN2; x2 = tgt2 + pos; x2T
                    for qt in range(NQT):
                        ps = psAT.tile([128, D], F32, tag="proj")
                        for h in range(NH):
                            nc.tensor.matmul(
                                out=ps[:],
                                lhsT=_r(oT[:, h, qt * 128:(qt + 1) * 128]),
                                rhs=_r(outw_s[:, h, :]),
                                start=(h == 0), stop=(h == NH - 1),
                                tile_position=(0, 0))
                        tgtb_t = work.tile([128, D], F32, tag="res_t")
                        nc.sync.dma_start(out=tgtb_t[:], in_=tgtb_own[qt])
                        r1 = work.tile([128, D], F32, tag="resid")
                        nc.vector.tensor_tensor(out=r1[:], in0=ps[:], in1=boutc_s[:],
                                                op=AO.add)
                        nc.vector.tensor_tensor(out=r1[:], in0=r1[:], in1=tgtb_t[:],
                                                op=AO.add)
                        _layernorm(nc, work, r1[:], tgt2[:, qt, :], ln2g_s, ln2b_s,
                                   eps_s)
                        if DBG:
                            nc.sync.dma_start(out=dbg[qt], in_=tgt2[:, qt, :])
                        for dt_ in range(2):
                            tp = psAT.tile([128, 128], F32, tag="sc")
                            nc.tensor.transpose(
                                out=tp[:], in_=tgt2[:, qt, dt_ * 128:(dt_ + 1) * 128],
                                identity=ident[:])
                            nc.vector.tensor_copy(
                                out=x2T[:, dt_, qt * 128:(qt + 1) * 128], in_=tp[:])

            # ---------------- deformable attention ----------------
            value_rows = value[:].rearrange("s (h d) -> (s h) d", h=NH)
            with (
                tc.tile_pool(name="pipe", bufs=1) as pipe,
                tc.tile_pool(name="gath", bufs=2) as gath,
                tc.tile_pool(name="psDF", bufs=2, space="PSUM") as psDF,
            ):
                for qt in range(NQT):
                    offp = psDF.tile([128, D], F32, tag="po")
                    for si, srcb in enumerate((x2T, poq_sb)):
                        for ki in range(2):
                            nc.tensor.matmul(
                                out=offp[:],
                                lhsT=srcb[:, ki, qt * 128:(qt + 1) * 128],
                                rhs=offw_s[:, ki, :],
                                start=(si == 0 and ki == 0),
                                stop=(si == 1 and ki == 1))
                    off_sb = pipe.tile([128, D], F32, tag="off_sb")
                    nc.vector.tensor_tensor(out=off_sb[:], in0=offp[:],
                                            in1=boffc_s[:], op=AO.add)
                    awp = psDF.tile([128, NH * 16], F32, tag="po")
                    for si, srcb in enumerate((x2T, poq_sb)):
                        for ki in range(2):
                            nc.tensor.matmul(
                                out=awp[:],
                                lhsT=srcb[:, ki, qt * 128:(qt + 1) * 128],
                                rhs=aww_s[:, ki, :],
                                start=(si == 0 and ki == 0),
                                stop=(si == 1 and ki == 1))
                    aw_sb = pipe.tile([128, NH * 16], F32, tag="aw_sb")
                    nc.vector.tensor_tensor(out=aw_sb[:], in0=awp[:], in1=bawc_s[:],
                                            op=AO.add)
                    if DBG:
                        nc.sync.dma_start(out=dbg3[qt], in_=off_sb[:])
                        nc.sync.dma_start(out=dbg4[qt], in_=aw_sb[:])
                    nc.scalar.activation(out=aw_sb[:], in_=aw_sb[:], func=AF.Exp)
                    awsum = pipe.tile([128, NH], F32, tag="awsum")
                    nc.vector.tensor_reduce(
                        out=awsum[:], in_=aw_sb[:].rearrange("p (h s) -> p h s", h=NH),
                        axis=mybir.AxisListType.X, op=AO.add)
                    nc.vector.reciprocal(out=awsum[:], in_=awsum[:])
                    nc.vector.tensor_tensor(
                        out=aw_sb[:], in0=aw_sb[:],
                        in1=_v(awsum[:], [list(awsum[:].ap[0]), [1, NH], [0, 16]]),
                        op=AO.mult)

                    ref_sb = pipe.tile([128, NL * 2], F32, tag="ref_sb")
                    nc.sync.dma_start(out=ref_sb[:], in_=ref_own[qt])
                    nc.vector.tensor_tensor(out=ref_sb[:], in0=ref_sb[:],
                                            in1=cwh_s[:, 0:NL * 2], op=AO.mult)
                    nc.vector.tensor_scalar(out=ref_sb[:], in0=ref_sb[:],
                                            scalar1=-0.5, scalar2=None, op0=AO.add)

                    # tap grid [128, 512] = (h, l, p, dy, dx)
                    p0o = list(off_sb[:].ap[0])
                    p0r = list(ref_sb[:].ap[0])
                    px = pipe.tile([128, TAPW], F32, tag="px")
                    py = pipe.tile([128, TAPW], F32, tag="py")
                    nc.vector.tensor_tensor(
                        out=px[:],
                        in0=_v(off_sb[:], [p0o, [32, NH], [2, 16], [0, 4]]),
                        in1=_v(ref_sb[:], [p0r, [0, NH], [2, NL], [0, 16]]),
                        op=AO.add)
                    nc.vector.tensor_tensor(
                        out=py[:],
                        in0=_v(off_sb[:], [p0o, [32, NH], [2, 16], [0, 4]], 1),
                        in1=_v(ref_sb[:], [p0r, [0, NH], [2, NL], [0, 16]], 1),
                        op=AO.add)

                    def floor_(x, tag):
                        ti = pipe.tile([128, TAPW], I32, tag="flr_i")
                        nc.vector.tensor_copy(out=ti[:], in_=x[:])
                        tf = pipe.tile([128, TAPW], F32, tag=tag)
                        nc.vector.tensor_copy(out=tf[:], in_=ti[:])
                        m_ = pipe.tile([128, TAPW], F32, tag="flr_m")
                        nc.vector.tensor_tensor(out=m_[:], in0=x[:], in1=tf[:],
                                                op=AO.is_lt)
                        nc.vector.tensor_tensor(out=tf[:], in0=tf[:], in1=m_[:],
                                                op=AO.subtract)
                        return tf

                    x0 = floor_(px, "fx")
                    y0 = floor_(py, "fy")
                    wx = pipe.tile([128, TAPW], F32, tag="wx")
                    wy = pipe.tile([128, TAPW], F32, tag="wy")
                    nc.vector.tensor_tensor(out=wx[:], in0=px[:], in1=x0[:],
                                            op=AO.subtract)
                    nc.vector.tensor_tensor(out=wy[:], in0=py[:], in1=y0[:],
                                            op=AO.subtract)
                    nc.vector.tensor_tensor(out=x0[:], in0=x0[:], in1=cDx_s[:],
                                            op=AO.add)
                    nc.vector.tensor_tensor(out=y0[:], in0=y0[:], in1=cDy_s[:],
                                            op=AO.add)

                    def wsel(w, cD, coord, cLim, tag):
                        onem = pipe.tile([128, TAPW], F32, tag="ws_o")
                        nc.vector.tensor_scalar(out=onem[:], in0=w[:], scalar1=-1.0,
                                                scalar2=1.0, op0=AO.mult, op1=AO.add)
                        sel = pipe.tile([128, TAPW], F32, tag=tag)
                        nc.vector.select(out=sel[:], mask=cD[:], on_true=w[:],
                                         on_false=onem[:])
                        v1 = pipe.tile([128, TAPW], F32, tag="ws_v")
                        nc.vector.tensor_scalar(out=v1[:], in0=coord[:], scalar1=0.0,
                                                scalar2=None, op0=AO.is_ge)
                        nc.vector.tensor_tensor(out=sel[:], in0=sel[:], in1=v1[:],
                                                op=AO.mult)
                        nc.vector.tensor_tensor(out=v1[:], in0=coord[:], in1=cLim[:],
                                                op=AO.is_lt)
                        nc.vector.tensor_tensor(out=sel[:], in0=sel[:], in1=v1[:],
                                                op=AO.mult)
                        return sel

                    wxt = wsel(wx, cDxu_s, x0, cW_s, "wxt")
                    wyt = wsel(wy, cDyu_s, y0, cH_s, "wyt")
                    wtap = pipe.tile([128, TAPW], F32, tag="wtap")
                    nc.vector.tensor_tensor(out=wtap[:], in0=wxt[:], in1=wyt[:],
                                            op=AO.mult)
                    nc.vector.tensor_tensor(
                        out=wtap[:], in0=wtap[:],
                        in1=_v(aw_sb[:], [list(aw_sb[:].ap[0]), [1, NH * 16], [0, 4]]),
                        op=AO.mult)

                    nc.vector.tensor_scalar(out=x0[:], in0=x0[:], scalar1=0.0,
                                            scalar2=None, op0=AO.max)
                    nc.vector.tensor_tensor(out=x0[:], in0=x0[:], in1=cWm1_s[:],
                                            op=AO.min)
                    nc.vector.tensor_scalar(out=y0[:], in0=y0[:], scalar1=0.0,
                                            scalar2=None, op0=AO.max)
                    nc.vector.tensor_tensor(out=y0[:], in0=y0[:], in1=cHm1_s[:],
                                            op=AO.min)
                    nc.vector.tensor_tensor(out=y0[:], in0=y0[:], in1=cW8_s[:],
                                            op=AO.mult)
                    nc.vector.scalar_tensor_tensor(out=y0[:], in0=x0[:], scalar=8.0,
                                                   in1=y0[:], op0=AO.mult, op1=AO.add)
                    nc.vector.tensor_tensor(out=y0[:], in0=y0[:], in1=cB8h_s[:],
                                            op=AO.add)
                    idx_i = pipe.tile([128, TAPW], I32, tag="idx_i")
                    nc.vector.tensor_copy(out=idx_i[:], in_=y0[:])
                    wb = pipe.tile([128, TAPW], BF16, tag="wb")
                    nc.vector.tensor_copy(out=wb[:], in_=wtap[:])

                    for h in range(NH):
                        g = gath.tile([128, NTAP, DH], BF16, tag="g")
                        for j in range(NTAP):
                            nc.gpsimd.indirect_dma_start(
                                out=g[:, j, :], out_offset=None, in_=value_rows,
                                in_offset=bass.IndirectOffsetOnAxis(
                                    ap=idx_i[:, h * NTAP + j:h * NTAP + j + 1],
                                    axis=0))
                        nc.vector.tensor_tensor(
                            out=g[:], in0=g[:],
                            in1=_v(wb[:],
                                   [list(wb[:].ap[0]), [1, NTAP], [0, DH]],
                                   h * NTAP),
                            op=AO.mult)
                        nc.vector.tensor_reduce(
                            out=oD[:, qt, h, :],
                            in_=_v(g[:], [list(g[:].ap[0]), [1, DH], [DH, NTAP]]),
                            axis=mybir.AxisListType.X, op=AO.add)

            if DBG:
                for qt in range(NQT):
                    nc.sync.dma_start(
                        out=dbg5[qt],
                        in_=oD[:, qt, :, :].rearrange("p h d -> p (h d)"))

            # ---------------- oproj + LN1 + FFN + LN3 ----------------
            with (
                tc.tile_pool(name="ffn", bufs=1) as ffn,
                tc.tile_pool(name="psFF", bufs=2, space="PSUM") as psFF,
            ):
                tgt3 = ffn.tile([128, NQT, D], F32)
                x3T = ffn.tile([128, 2, QPAD], BF16)
                ff1T = ffn.tile([128, DFFN // 128, QPAD], BF16)
                for qt in range(NQT):
                    oTd = work.tile([32, NH, 128], BF16, tag="oTd")
                    for h in range(NH):
                        tp = psFF.tile([32, 128], F32, tag="tp2")
                        nc.tensor.transpose(out=tp[:], in_=oD[:, qt, h, :],
                                            identity=ident[:])
                        nc.vector.tensor_copy(out=oTd[:, h, :], in_=tp[:])
                    ps = psFF.tile([128, D], F32, tag="op2")
                    for h in range(NH):
                        nc.tensor.matmul(
                            out=ps[:], lhsT=_r(oTd[:, h, :]), rhs=_r(opw_s[:, h, :]),
                            start=(h == 0), stop=(h == NH - 1), tile_position=(0, 0))
                    r2 = work.tile([128, D], F32, tag="resid")
                    nc.vector.tensor_tensor(out=r2[:], in0=ps[:], in1=bopc_s[:],
                                            op=AO.add)
                    nc.vector.tensor_tensor(out=r2[:], in0=r2[:], in1=tgt2[:, qt, :],
                                            op=AO.add)
                    _layernorm(nc, work, r2[:], tgt3[:, qt, :], ln1g_s, ln1b_s, eps_s)
                    if DBG:
                        nc.sync.dma_start(out=dbg2[qt], in_=tgt3[:, qt, :])
                    for dt_ in range(2):
                        tp = psFF.tile([128, 128], F32, tag="tp3")
                        nc.tensor.transpose(
                            out=tp[:], in_=tgt3[:, qt, dt_ * 128:(dt_ + 1) * 128],
                            identity=ident[:])
                        nc.vector.tensor_copy(
                            out=x3T[:, dt_, qt * 128:(qt + 1) * 128], in_=tp[:])

                for ft in range(DFFN // 128):
                    ps = psFF.tile([128, QPAD], F32, tag="ff1")
                    for ki in range(2):
                        nc.tensor.matmul(
                            out=ps[:], lhsT=_r(l1w_s[:, ki, ft * 128:(ft + 1) * 128]),
                            rhs=_r(x3T[:, ki, :]), start=(ki == 0), stop=(ki == 1))
                    nc.scalar.activation(out=ff1T[:, ft, :], in_=ps[:], func=AF.Relu,
                                         bias=b1col_s[:, ft:ft + 1], scale=1.0)

                for qt in range(NQT):
                    ps = psFF.tile([128, D], F32, tag="op2")
                    for ft in range(DFFN // 128):
                        nc.tensor.matmul(
                            out=ps[:], lhsT=_r(ff1T[:, ft, qt * 128:(qt + 1) * 128]),
                            rhs=_r(l2w_s[:, ft, :]),
                            start=(ft == 0), stop=(ft == DFFN // 128 - 1))
                    r3 = work.tile([128, D], F32, tag="resid")
                    nc.vector.tensor_tensor(out=r3[:], in0=ps[:], in1=b2c_s[:],
                                            op=AO.add)
                    nc.vector.tensor_tensor(out=r3[:], in0=r3[:], in1=tgt3[:, qt, :],
                                            op=AO.add)
                    o_sb = work.tile([128, D], F32, tag="o_sb")
                    _layernorm(nc, work, r3[:], o_sb[:], ln3g_s, ln3b_s, eps_s)
                    nc.sync.dma_start(out=out[qt], in_=o_sb[:])

    nc.compile()
    return nc


_NC_CACHE = None


def _get_nc():
    global _NC_CACHE
    if _NC_CACHE is None:
        _NC_CACHE = build_program()
    return _NC_CACHE


BF16NP = ml_dtypes.bfloat16


def _kt(w, dt=BF16NP):
    """(256, X) -> [128, 2, X] K-tiled SBUF layout."""
    return np.ascontiguousarray(w.reshape(2, 128, -1).transpose(1, 0, 2)).astype(dt)


def _host_prep(inputs):
    f = np.float32
    tgt = np.asarray(inputs["tgt"], f)
    pos = np.asarray(inputs["tgt_query_pos"], f)
    ref = np.asarray(inputs["tgt_reference_points"], f)
    mem = np.asarray(inputs["memory"], f)

    ipw = np.asarray(inputs["in_proj_w"], f); ipb = np.asarray(inputs["in_proj_b"], f)
    sc = 1.0 / math.sqrt(DH)
    shared = dict(
        wqT=_kt(ipw[0:D].T * sc), wkT=_kt(ipw[D:2 * D].T), wvT=_kt(ipw[2 * D:3 * D].T),
        bqp=np.ascontiguousarray((ipb[0:D] * sc).reshape(2, 128).T),
        bkp=np.ascontiguousarray(ipb[D:2 * D].reshape(2, 128).T),
        bvc=ipb[2 * D:3 * D][None],
        outwT8=np.ascontiguousarray(
            np.asarray(inputs["out_proj_w"], f).T.reshape(NH, 32, D)
            .transpose(1, 0, 2)).reshape(32, NH * D).astype(BF16NP),
        boutc=np.asarray(inputs["out_proj_b"], f)[None],
        vprojwT=_kt(np.asarray(inputs["vproj_w"], f).T),
        bvpc=np.asarray(inputs["vproj_b"], f)[None],
        offwT=_kt(np.asarray(inputs["off_w"], f).T),
        boffc=np.asarray(inputs["off_b"], f)[None],
        awwT=_kt(np.asarray(inputs["aw_w"], f).T),
        bawc=np.asarray(inputs["aw_b"], f)[None],
        oprojwT8=np.ascontiguousarray(
            np.asarray(inputs["oproj_w"], f).T.reshape(NH, 32, D)
            .transpose(1, 0, 2)).reshape(32, NH * D).astype(BF16NP),
        bopc=np.asarray(inputs["oproj_b"], f)[None],
        lin1wT=_kt(np.asarray(inputs["lin1_w"], f).T),
        b1col=np.ascontiguousarray(
            np.asarray(inputs["lin1_b"], f).reshape(DFFN // 128, 128).T),
        lin2wT=np.ascontiguousarray(
            np.asarray(inputs["lin2_w"], f).T.reshape(DFFN // 128, 128, D)
            .transpose(1, 0, 2)).astype(BF16NP),
        b2c=np.asarray(inputs["lin2_b"], f)[None],
        ln2g=np.asarray(inputs["ln2_g"], f)[None], ln2b=np.asarray(inputs["ln2_b"], f)[None],
        ln1g=np.asarray(inputs["ln1_g"], f)[None], ln1b=np.asarray(inputs["ln1_b"], f)[None],
        ln3g=np.asarray(inputs["ln3_g"], f)[None], ln3b=np.asarray(inputs["ln3_b"], f)[None],
    )

    # tap-grid constants: slot t = l*16 + p*4 + dy*2 + dx, tiled over h
    t = np.arange(NTAP)
    lv = t >> 4; dy = (t >> 1) & 1; dx = t & 1
    Wl = np.array([SPATIAL[i][1] for i in range(NL)], f)[lv]
    Hl = np.array([SPATIAL[i][0] for i in range(NL)], f)[lv]
    base = np.array([LEVEL_START[i] for i in range(NL)], f)[lv]
    shared.update(
        cDx=np.tile(dx.astype(f), NH)[None], cDy=np.tile(dy.astype(f), NH)[None],
        cDxu=np.tile(dx.astype(np.uint8), NH)[None],
        cDyu=np.tile(dy.astype(np.uint8), NH)[None],
        cW=np.tile(Wl, NH)[None], cH=np.tile(Hl, NH)[None],
        cWm1=np.tile(Wl - 1, NH)[None], cHm1=np.tile(Hl - 1, NH)[None],
        cW8=np.tile(Wl * 8, NH)[None],
        cB8h=(np.tile(base * 8, NH)
              + np.repeat(np.arange(NH, dtype=f), NTAP))[None],
    )
    cwh = np.empty((1, NL * 2), f)
    for i, (H, W) in enumerate(SPATIAL):
        cwh[0, 2 * i] = W; cwh[0, 2 * i + 1] = H
    shared["cwh"] = cwh

    in_maps = []
    for c in range(8):
        b, half = c // 2, c % 2
        q0 = half * QH
        tgtbT = np.zeros((D, NKPAD), f); tgtbT[:, :NQ] = tgt[:, b, :].T
        posbT = np.zeros((D, NKPAD), f); posbT[:, :NQ] = pos[:, b, :].T
        tgtb_ownT = np.zeros((D, QPAD), f); tgtb_ownT[:, :QH] = tgt[q0:q0 + QH, b, :].T
        posb_ownT = np.zeros((D, QPAD), f); posb_ownT[:, :QH] = pos[q0:q0 + QH, b, :].T
        tgtb_own = np.zeros((QPAD, D), f); tgtb_own[:QH] = tgt[q0:q0 + QH, b, :]
        pos_own = np.zeros((QPAD, D), f); pos_own[:QH] = pos[q0:q0 + QH, b, :]
        ref_own = np.zeros((QPAD, NL * 2), f)
        ref_own[:QH] = ref[q0:q0 + QH, b].reshape(QH, NL * 2)
        memTb = np.zeros((D, SPAD), f); memTb[:, :S] = mem[:, b, :].T

        def t3(x, w):  # (256, W) -> [128, 2, W]
            return np.ascontiguousarray(x.reshape(2, 128, w).transpose(1, 0, 2))

        m = dict(shared)
        m.update(
            tgtbT=t3(tgtbT, NKPAD).astype(BF16NP),
            posbT=t3(posbT, NKPAD).astype(BF16NP),
            tgtb_ownT=t3(tgtb_ownT, QPAD).astype(BF16NP),
            posb_ownT=t3(posb_ownT, QPAD).astype(BF16NP),
            tgtb_own=tgtb_own.reshape(NQT, 128, D),
            pos_own=pos_own.reshape(NQT, 128, D),
            ref_own=ref_own.reshape(NQT, 128, NL * 2),
            memT=t3(memTb, SPAD).astype(BF16NP),
        )
        in_maps.append(m)
    return in_maps


def kernel(**inputs):
    nc = _get_nc()
    in_maps = _host_prep(inputs)
    res = run_bass_kernel_spmd(nc, in_maps, list(range(8))).results
    outp = np.empty((NQ, BS, D), np.float32)
    for c in range(8):
        b, half = c // 2, c % 2
        q0 = half * QH
        o = np.asarray(res[c]["out"], np.float32).reshape(QPAD, D)
        outp[q0:q0 + QH, b, :] = o[:QH]
    return outp

